# revision 11
# baseline (speedup 1.0000x reference)
"""Group VQ (vq_codebook) Trainium2 Bass kernel.

Strategy: data-parallel over batch B=16 across 8 cores (2 batches/core).
Per core, per (group g, batch b, 125-token tile): two fp16 matmuls
[66,125]x[66,512] -> PSUM [125,1024] compute scores
s[t,k] = 2*x_t.e_k - |e_k|^2 (x rows 0..63 = fp16(x); rows 64,65 = 1.0;
E rows 64,65 carry -|e|^2 split hi/lo in fp16 so e2 is fp32-exact; the
single fp16 product term has ~6e-3 score noise, rescued on the host).
The 1024-code argmax is reduced on-device to 16 interleaved segment
maxima (segment s = codes [32s:32s+32] U [512+32s:512+32s+32]).
The reduce is DVE-only (TRN2: GPSIMD has no PSUM access and no
TensorTensor; no 2x DVE modes for reduce ops), so the Act engine
stages 4 tiles of PSUM scores into one SBUF tile (cheaper DVE reads,
frees PSUM early) and DVE does one batched 5-level-AP segmented max
per 4 tiles. Modeled per-core busy: DVE ~565 us (bound), Act ~539 us,
PE ~230 us (1024 matmuls); total ~574 us vs ~678 us for the previous
3-term-fp16 + per-tile-reduce design.

Host: picks the top-2 segments per token from the device maxima and
rescores their 2x64 candidate codes exactly in fp32 via per-(group,
segment) batched sgemm, then gathers code vectors. The top-2 rescue
makes the fp16 score noise harmless: a wrong final pick needs a 3-way
cross-segment near-tie, so the end-to-end mismatch set equals the
reference's own fp32 near-ties (rel ~1.8e-3, threshold 2e-2).
"""
import sys
import numpy as np
from contextlib import ExitStack

sys.path.insert(0, "/opt/trn_rl_repo")

B, C, F, T = 16, 2, 256, 4000
G, K, D = 8, 1024, 64
NCORES = 8
NB = B // NCORES          # batches per core = 2
TT = 125                  # tokens per tile (4000 = 32*125)
ST = 2000                 # tokens per x-DMA supertile (16 tiles)
NTILES = T // TT          # 32
NSUP = T // ST            # 2
TPS = ST // TT            # tiles per supertile = 16
NSEG = 16                 # segments per 1024 codes
SEGW = K // NSEG          # 64 candidate codes per segment (interleaved)
XR = D + 2                # x rows: 64 features + two ones rows = 66

_compiled = None


def _build_program():
    import concourse.bass as bass
    import concourse.tile as tile
    from concourse import bacc, mybir

    nc = bacc.Bacc(
        "TRN2",
        target_bir_lowering=False,
        debug=False,
        enable_asserts=False,
        num_devices=NCORES,
    )
    f32 = mybir.dt.float32
    f16 = mybir.dt.float16
    xa = nc.dram_tensor("xa", [NB, G, XR, T], f16, kind="ExternalInput").ap()
    et = nc.dram_tensor("et", [G, XR, K], f16, kind="ExternalInput").ap()
    om = nc.dram_tensor(
        "om", [G * NB, TT, NTILES * NSEG], f32, kind="ExternalOutput"
    ).ap()

    with tile.TileContext(nc) as tc, ExitStack() as ctx:
        epool = ctx.enter_context(tc.tile_pool(name="e", bufs=1))
        xpool = ctx.enter_context(tc.tile_pool(name="x", bufs=3))
        ppool = ctx.enter_context(
            tc.tile_pool(name="ps", bufs=2, space=bass.MemorySpace.PSUM)
        )
        spool = ctx.enter_context(tc.tile_pool(name="scp", bufs=3))
        mpool = ctx.enter_context(tc.tile_pool(name="mseg", bufs=2))

        etiles = []
        for g in range(G):
            e_t = epool.tile([XR, K], f16, tag=f"e{g}")
            nc.sync.dma_start(e_t[:], et[g])
            etiles.append(e_t)

        for g in range(G):
            for b in range(NB):
                m_sb = mpool.tile([TT, NTILES * NSEG], f32)
                for s in range(NSUP):
                    xt = xpool.tile([XR, ST], f16, tag="x")
                    nc.sync.dma_start(xt[:], xa[b, g, :, s * ST:(s + 1) * ST])
                    for quad in range(TPS // 4):
                        # four token-tiles per DVE reduce; two PSUM groups
                        # (4 banks each) staged into one SBUF tile by Act.
                        scp = spool.tile([TT, 4 * K], f32)
                        for pr in range(2):
                            ps = ppool.tile([TT, 2 * K], f32)
                            for h in range(2):
                                k = quad * 4 + pr * 2 + h
                                xsl = xt[:, k * TT:(k + 1) * TT]
                                nc.tensor.matmul(ps[:, h * K:h * K + K // 2],
                                                 xsl, etiles[g][:, :K // 2],
                                                 start=True, stop=True)
                                nc.tensor.matmul(
                                    ps[:, h * K + K // 2:(h + 1) * K],
                                    xsl, etiles[g][:, K // 2:],
                                    start=True, stop=True)
                            # Act stages scores to SBUF: cheaper DVE reads
                            # and frees PSUM sooner (GPSIMD/Pool cannot
                            # help: no PSUM access, no TensorTensor).
                            nc.scalar.activation(
                                scp[:, pr * 2 * K:(pr + 1) * 2 * K], ps[:],
                                mybir.ActivationFunctionType.Copy)
                        tloc = s * TPS + quad * 4
                        # interleaved segments: seg s of each tile = max over
                        # codes [32s:32s+32] u [512+32s:512+32s+32]
                        nc.vector.tensor_reduce(
                            m_sb[:, tloc * NSEG:(tloc + 4) * NSEG],
                            scp[:].rearrange("p (t h s w) -> p t s h w",
                                             t=4, h=2, s=NSEG, w=32),
                            axis=mybir.AxisListType.XY,
                            op=mybir.AluOpType.max,
                        )
                nc.sync.dma_start(om[g * NB + b], m_sb[:])

    nc.compile()
    return nc


def _get_compiled():
    global _compiled
    if _compiled is None:
        _compiled = _build_program()
    return _compiled


def _prep_inputs(x, codebooks):
    # xa: [B, G, 66, T] fp16 — rows 0..63 = fp16(x), rows 64,65 = 1.0
    xg = x.reshape(B, G, D, T)
    xa = np.empty((B, G, XR, T), dtype=np.float16)
    xa[:, :, :D] = xg
    xa[:, :, D:] = 1.0
    # et: [G, 66, K] fp16 — rows 0..63 = 2*E^T; rows 64,65 = -|e|^2 hi/lo
    e2 = (codebooks.astype(np.float32) ** 2).sum(-1)          # [G, K]
    eh = (-e2).astype(np.float16)
    el = (-e2 - eh.astype(np.float32)).astype(np.float16)
    et = np.empty((G, XR, K), dtype=np.float16)
    et[:, :D] = 2.0 * np.transpose(codebooks, (0, 2, 1))
    et[:, D] = eh
    et[:, D + 1] = el
    return xa, et


def run_device(x, codebooks, trace=False):
    from concourse.bass_utils import run_bass_kernel_spmd

    nc = _get_compiled()
    xa, et = _prep_inputs(np.asarray(x, np.float32),
                          np.asarray(codebooks, np.float32))
    in_maps = []
    for core in range(NCORES):
        sl = slice(core * NB, (core + 1) * NB)
        in_maps.append({"xa": np.ascontiguousarray(xa[sl]), "et": et})
    res = run_bass_kernel_spmd(nc, in_maps, list(range(NCORES)), trace=trace)
    return res


# candidate code indices per interleaved segment: [NSEG, SEGW]
_CAND = np.concatenate(
    [np.arange(32)[None, :] + 32 * np.arange(NSEG)[:, None],
     512 + np.arange(32)[None, :] + 32 * np.arange(NSEG)[:, None]], axis=1)


def _host_finish(x, codebooks, m16):
    """m16: [G, B, T, NSEG] fp32 device segment maxima.
    Rescore the top-2 segments' 2*64 candidates exactly in fp32."""
    xg = x.reshape(B, G, D, T)
    # tokens as [G, B*T, D]
    tok = np.ascontiguousarray(
        np.transpose(xg, (1, 0, 3, 2)).reshape(G, B * T, D))
    m2 = m16.reshape(G, B * T, NSEG)
    # top-2 segments per token
    s1 = np.argmax(m2, axis=-1)                               # [G, N]
    m2m = np.copy(m2)
    np.put_along_axis(m2m, s1[..., None], -np.inf, axis=-1)
    s2 = np.argmax(m2m, axis=-1)                              # [G, N]
    out = np.empty((B, G, D, T), dtype=np.float32)
    n = B * T
    for g in range(G):
        cb = codebooks[g].astype(np.float32)                  # [K, D]
        e2 = (cb * cb).sum(-1)                                # [K]
        w = 2.0 * cb.T                                        # [D, K]
        best_val = np.full(n, -np.inf, dtype=np.float32)
        best_idx = np.zeros(n, dtype=np.int64)
        for seg in range(NSEG):
            cand = _CAND[seg]                                 # [64]
            mask = (s1[g] == seg) | (s2[g] == seg)
            rows = np.nonzero(mask)[0]
            if rows.size == 0:
                continue
            a = tok[g][rows]                                  # [N_s, D]
            sc = a @ w[:, cand] - e2[cand]                    # [N_s, 64]
            loc = np.argmax(sc, axis=1)
            val = sc[np.arange(rows.size), loc]
            idx = cand[loc]
            upd = (val > best_val[rows]) | (
                (val == best_val[rows]) & (idx < best_idx[rows]))
            r_upd = rows[upd]
            best_val[r_upd] = val[upd]
            best_idx[r_upd] = idx[upd]
        q = cb[best_idx]                                      # [N, D]
        out[:, g] = q.reshape(B, T, D).transpose(0, 2, 1)
    return out.reshape(B, C, F, T)


def kernel(x, codebooks):
    x = np.asarray(x, dtype=np.float32)
    codebooks = np.asarray(codebooks, dtype=np.float32)
    res = run_device(x, codebooks)
    # om [G*NB, TT, NTILES*NSEG]; token t = tloc*TT + p
    m16 = np.empty((G, B, T, NSEG), dtype=np.float32)
    for core in range(NCORES):
        o = res.results[core]["om"].reshape(G, NB, TT, NTILES, NSEG)
        m16[:, core * NB:(core + 1) * NB] = o.transpose(0, 1, 3, 2, 4).reshape(
            G, NB, T, NSEG
        )
    q = _host_finish(x, codebooks, m16)
    x_q = x + (q - x)
    return x_q, q


# revision 14
# speedup vs baseline: 1.0928x; 1.0928x over previous
"""Group VQ (vq_codebook) Trainium2 Bass kernel.

Strategy: data-parallel over batch B=16 across 8 cores (2 batches/core).
Per core, per (group g, batch b, 125-token tile): two fp16 matmuls
[66,125]x[66,512] -> PSUM [125,1024] compute scores
s[t,k] = 2*x_t.e_k - |e_k|^2 (x rows 0..63 = fp16(x); rows 64,65 = 1.0;
E rows 64,65 carry -|e|^2 split hi/lo in fp16 so e2 is fp32-exact; the
single fp16 product term has ~6e-3 score noise, rescued on the host).

The 1024-code argmax is reduced on-device to 16 segment maxima per
tile, where segment s = the stride-16 residue class {s + 16*i}. Reduce
ops (tensor_reduce/max/pool_max) have no DVE fast modes, but fp16
elementwise tensor_tensor runs at 2x — so the reduction is a binary
tree of batched fp16 tensor_max folds (1024->512->...->16), which
models at ~758 ns per tile vs 1104 for a tensor_reduce. Most quads
(4-tile groups) are staged PSUM->SBUF-fp16 by the otherwise-idle Act
engine; ~19% of quads skip Act and run the first fold straight from
PSUM on DVE (slower for DVE, but it rebalances Act vs DVE to ~445
us/core each). PE: 1024 matmuls = ~230 us/core. Previous designs:
677 us (3-term fp16 + per-tile tensor_reduce), 574 us (1-term fp16 +
Act staging + batched tensor_reduce).

Host: picks the top-2 segments per token from the device maxima and
rescores their 2x64 candidate codes exactly in fp32 via per-(group,
segment) batched sgemm, then gathers code vectors. The top-2 rescue
makes fp16 score/seg-max noise harmless: a wrong final pick needs a
3-way cross-segment near-tie, so end-to-end mismatches stay at the
fp32 reference's own near-tie level (rel ~2e-3, threshold 2e-2).
"""
import sys
import numpy as np
from contextlib import ExitStack

sys.path.insert(0, "/opt/trn_rl_repo")

B, C, F, T = 16, 2, 256, 4000
G, K, D = 8, 1024, 64
NCORES = 8
NB = B // NCORES          # batches per core = 2
TT = 125                  # tokens per tile (4000 = 32*125)
ST = 2000                 # tokens per x-DMA supertile (16 tiles)
NTILES = T // TT          # 32
NSUP = T // ST            # 2
TPS = ST // TT            # tiles per supertile = 16
NSEG = 16                 # segments per 1024 codes
SEGW = K // NSEG          # 64 candidate codes per segment (stride-16)
XR = D + 2                # x rows: 64 features + two ones rows = 66

_compiled = None


def _build_program():
    import concourse.bass as bass
    import concourse.tile as tile
    from concourse import bacc, mybir

    nc = bacc.Bacc(
        "TRN2",
        target_bir_lowering=False,
        debug=False,
        enable_asserts=False,
        num_devices=NCORES,
    )
    f32 = mybir.dt.float32
    f16 = mybir.dt.float16
    xa = nc.dram_tensor("xa", [NB, G, XR, T], f16, kind="ExternalInput").ap()
    et = nc.dram_tensor("et", [G, XR, K], f16, kind="ExternalInput").ap()
    om = nc.dram_tensor(
        "om", [G * NB, TT, NTILES * NSEG], f16, kind="ExternalOutput"
    ).ap()

    def fold(out_flat, in_flat, t, w):
        """Elementwise fp16 max of the two halves of each w-wide block:
        in [p, t*w] -> out [p, t*(w/2)]; pairs element j with j+w/2."""
        ri = in_flat.rearrange("p (t k) -> p t k", t=t, k=w)
        ro = out_flat.rearrange("p (t k) -> p t k", t=t, k=w // 2)
        nc.vector.tensor_max(ro, ri[:, :, :w // 2], ri[:, :, w // 2:])

    with tile.TileContext(nc) as tc, ExitStack() as ctx:
        epool = ctx.enter_context(tc.tile_pool(name="e", bufs=1))
        xpool = ctx.enter_context(tc.tile_pool(name="x", bufs=3))
        ppool = ctx.enter_context(
            tc.tile_pool(name="ps", bufs=2, space=bass.MemorySpace.PSUM)
        )
        spool = ctx.enter_context(tc.tile_pool(name="scp", bufs=3))
        tpool = ctx.enter_context(tc.tile_pool(name="tree", bufs=2))
        mpool = ctx.enter_context(tc.tile_pool(name="mseg", bufs=2))

        etiles = []
        for g in range(G):
            e_t = epool.tile([XR, K], f16, tag=f"e{g}")
            nc.sync.dma_start(e_t[:], et[g])
            etiles.append(e_t)

        for g in range(G):
            for b in range(NB):
                gb = g * NB + b
                # 1 of 8 quads runs DVE-direct from PSUM to offload Act
                direct = {3}
                m_sb = mpool.tile([TT, NTILES * NSEG], f16)
                for s in range(NSUP):
                    xt = xpool.tile([XR, ST], f16, tag="x")
                    nc.sync.dma_start(xt[:], xa[b, g, :, s * ST:(s + 1) * ST])
                    for quad in range(TPS // 4):
                        qg = s * (TPS // 4) + quad
                        tbase = s * TPS + quad * 4
                        if qg not in direct:
                            # Route A: Act stages 4 tiles of PSUM scores to
                            # SBUF fp16, DVE folds a t=4-batched tree.
                            scp = spool.tile([TT, 4 * K], f16)
                            for pr in range(2):
                                ps = ppool.tile([TT, 2 * K], f32)
                                for h in range(2):
                                    k = quad * 4 + pr * 2 + h
                                    xsl = xt[:, k * TT:(k + 1) * TT]
                                    nc.tensor.matmul(
                                        ps[:, h * K:h * K + K // 2], xsl,
                                        etiles[g][:, :K // 2],
                                        start=True, stop=True)
                                    nc.tensor.matmul(
                                        ps[:, h * K + K // 2:(h + 1) * K],
                                        xsl, etiles[g][:, K // 2:],
                                        start=True, stop=True)
                                nc.scalar.activation(
                                    scp[:, pr * 2 * K:(pr + 1) * 2 * K],
                                    ps[:], mybir.ActivationFunctionType.Copy)
                            l1 = tpool.tile([TT, 2048], f16, tag="l1")
                            fold(l1[:], scp[:], 4, 1024)
                            l2 = tpool.tile([TT, 1024], f16, tag="l2")
                            fold(l2[:], l1[:], 4, 512)
                            l3 = tpool.tile([TT, 512], f16, tag="l3")
                            fold(l3[:], l2[:], 4, 256)
                            l4 = tpool.tile([TT, 256], f16, tag="l4")
                            fold(l4[:], l3[:], 4, 128)
                            l5 = tpool.tile([TT, 128], f16, tag="l5")
                            fold(l5[:], l4[:], 4, 64)
                            fold(m_sb[:, tbase * NSEG:(tbase + 4) * NSEG],
                                 l5[:], 4, 32)
                        else:
                            # Route B: DVE folds straight from PSUM (f32 in,
                            # f16 out) per 2-tile pair; no Act involvement.
                            for pr in range(2):
                                ps = ppool.tile([TT, 2 * K], f32)
                                for h in range(2):
                                    k = quad * 4 + pr * 2 + h
                                    xsl = xt[:, k * TT:(k + 1) * TT]
                                    nc.tensor.matmul(
                                        ps[:, h * K:h * K + K // 2], xsl,
                                        etiles[g][:, :K // 2],
                                        start=True, stop=True)
                                    nc.tensor.matmul(
                                        ps[:, h * K + K // 2:(h + 1) * K],
                                        xsl, etiles[g][:, K // 2:],
                                        start=True, stop=True)
                                # DVE TensorTensor may read only one PSUM
                                # input: stage upper halves to SBUF first.
                                ri = ps[:].rearrange("p (t k) -> p t k",
                                                     t=2, k=K)
                                s16u = tpool.tile([TT, 1024], f16, tag="su")
                                rs = s16u[:].rearrange("p (t k) -> p t k",
                                                       t=2, k=K // 2)
                                nc.vector.tensor_copy(rs, ri[:, :, K // 2:])
                                b1 = tpool.tile([TT, 1024], f16, tag="b1")
                                nc.vector.tensor_max(
                                    b1[:].rearrange("p (t k) -> p t k",
                                                    t=2, k=K // 2),
                                    ri[:, :, :K // 2], rs)
                                b2 = tpool.tile([TT, 512], f16, tag="b2")
                                fold(b2[:], b1[:], 2, 512)
                                b3 = tpool.tile([TT, 256], f16, tag="b3")
                                fold(b3[:], b2[:], 2, 256)
                                b4 = tpool.tile([TT, 128], f16, tag="b4")
                                fold(b4[:], b3[:], 2, 128)
                                b5 = tpool.tile([TT, 64], f16, tag="b5")
                                fold(b5[:], b4[:], 2, 64)
                                t0 = tbase + pr * 2
                                fold(m_sb[:, t0 * NSEG:(t0 + 2) * NSEG],
                                     b5[:], 2, 32)
                nc.sync.dma_start(om[gb], m_sb[:])

    nc.compile()
    return nc


def _get_compiled():
    global _compiled
    if _compiled is None:
        _compiled = _build_program()
    return _compiled


def _prep_inputs(x, codebooks):
    # xa: [B, G, 66, T] fp16 — rows 0..63 = fp16(x), rows 64,65 = 1.0
    xg = x.reshape(B, G, D, T)
    xa = np.empty((B, G, XR, T), dtype=np.float16)
    xa[:, :, :D] = xg
    xa[:, :, D:] = 1.0
    # et: [G, 66, K] fp16 — rows 0..63 = 2*E^T; rows 64,65 = -|e|^2 hi/lo
    e2 = (codebooks.astype(np.float32) ** 2).sum(-1)          # [G, K]
    eh = (-e2).astype(np.float16)
    el = (-e2 - eh.astype(np.float32)).astype(np.float16)
    et = np.empty((G, XR, K), dtype=np.float16)
    et[:, :D] = 2.0 * np.transpose(codebooks, (0, 2, 1))
    et[:, D] = eh
    et[:, D + 1] = el
    return xa, et


def run_device(x, codebooks, trace=False):
    from concourse.bass_utils import run_bass_kernel_spmd

    nc = _get_compiled()
    xa, et = _prep_inputs(np.asarray(x, np.float32),
                          np.asarray(codebooks, np.float32))
    in_maps = []
    for core in range(NCORES):
        sl = slice(core * NB, (core + 1) * NB)
        in_maps.append({"xa": np.ascontiguousarray(xa[sl]), "et": et})
    res = run_bass_kernel_spmd(nc, in_maps, list(range(NCORES)), trace=trace)
    return res


# candidate code indices per segment: stride-16 residue classes [NSEG, SEGW]
_CAND = np.arange(NSEG)[:, None] + NSEG * np.arange(SEGW)[None, :]


def _host_finish(x, codebooks, m16):
    """m16: [G, B, T, NSEG] fp32 device segment maxima.
    Rescore the top-2 segments' 2*64 candidates exactly in fp32."""
    xg = x.reshape(B, G, D, T)
    # tokens as [G, B*T, D]
    tok = np.ascontiguousarray(
        np.transpose(xg, (1, 0, 3, 2)).reshape(G, B * T, D))
    m2 = m16.reshape(G, B * T, NSEG)
    # top-2 segments per token
    s1 = np.argmax(m2, axis=-1)                               # [G, N]
    m2m = np.copy(m2)
    np.put_along_axis(m2m, s1[..., None], -np.inf, axis=-1)
    s2 = np.argmax(m2m, axis=-1)                              # [G, N]
    out = np.empty((B, G, D, T), dtype=np.float32)
    n = B * T
    for g in range(G):
        cb = codebooks[g].astype(np.float32)                  # [K, D]
        e2 = (cb * cb).sum(-1)                                # [K]
        w = 2.0 * cb.T                                        # [D, K]
        best_val = np.full(n, -np.inf, dtype=np.float32)
        best_idx = np.zeros(n, dtype=np.int64)
        for seg in range(NSEG):
            cand = _CAND[seg]                                 # [64]
            mask = (s1[g] == seg) | (s2[g] == seg)
            rows = np.nonzero(mask)[0]
            if rows.size == 0:
                continue
            a = tok[g][rows]                                  # [N_s, D]
            sc = a @ w[:, cand] - e2[cand]                    # [N_s, 64]
            loc = np.argmax(sc, axis=1)
            val = sc[np.arange(rows.size), loc]
            idx = cand[loc]
            upd = (val > best_val[rows]) | (
                (val == best_val[rows]) & (idx < best_idx[rows]))
            r_upd = rows[upd]
            best_val[r_upd] = val[upd]
            best_idx[r_upd] = idx[upd]
        q = cb[best_idx]                                      # [N, D]
        out[:, g] = q.reshape(B, T, D).transpose(0, 2, 1)
    return out.reshape(B, C, F, T)


def kernel(x, codebooks):
    x = np.asarray(x, dtype=np.float32)
    codebooks = np.asarray(codebooks, dtype=np.float32)
    res = run_device(x, codebooks)
    # om [G*NB, TT, NTILES*NSEG] fp16; token t = tloc*TT + p
    m16 = np.empty((G, B, T, NSEG), dtype=np.float32)
    for core in range(NCORES):
        o = res.results[core]["om"].astype(np.float32).reshape(
            G, NB, TT, NTILES, NSEG)
        m16[:, core * NB:(core + 1) * NB] = o.transpose(0, 1, 3, 2, 4).reshape(
            G, NB, T, NSEG
        )
    q = _host_finish(x, codebooks, m16)
    x_q = x + (q - x)
    return x_q, q


# revision 15
# speedup vs baseline: 1.0964x; 1.0033x over previous
"""Group VQ (vq_codebook) Trainium2 Bass kernel.

Strategy: data-parallel over batch B=16 across 8 cores (2 batches/core).
Per core, per (group g, batch b, 125-token tile): two fp16 matmuls
[66,125]x[66,512] -> PSUM [125,1024] compute scores
s[t,k] = 2*x_t.e_k - |e_k|^2 (x rows 0..63 = fp16(x); rows 64,65 = 1.0;
E rows 64,65 carry -|e|^2 split hi/lo in fp16 so e2 is fp32-exact; the
single fp16 product term has ~6e-3 score noise, rescued on the host).

The 1024-code argmax is reduced on-device to 16 segment maxima per
tile, where segment s = the stride-16 residue class {s + 16*i}. Reduce
ops (tensor_reduce/max/pool_max) have no DVE fast modes, but fp16
elementwise tensor_tensor runs at 2x — so the reduction is a binary
tree of batched fp16 tensor_max folds (1024->512->...->16), which
models at ~758 ns per tile vs 1104 for a tensor_reduce. Most quads
(4-tile groups) are staged PSUM->SBUF-fp16 by the otherwise-idle Act
engine; ~19% of quads skip Act and run the first fold straight from
PSUM on DVE (slower for DVE, but it rebalances Act vs DVE to ~445
us/core each). PE: 1024 matmuls = ~230 us/core. Previous designs:
677 us (3-term fp16 + per-tile tensor_reduce), 574 us (1-term fp16 +
Act staging + batched tensor_reduce).

Host: picks the top-2 segments per token from the device maxima and
rescores their 2x64 candidate codes exactly in fp32 via per-(group,
segment) batched sgemm, then gathers code vectors. The top-2 rescue
makes fp16 score/seg-max noise harmless: a wrong final pick needs a
3-way cross-segment near-tie, so end-to-end mismatches stay at the
fp32 reference's own near-tie level (rel ~2e-3, threshold 2e-2).
"""
import sys
import numpy as np
from contextlib import ExitStack

sys.path.insert(0, "/opt/trn_rl_repo")

B, C, F, T = 16, 2, 256, 4000
G, K, D = 8, 1024, 64
NCORES = 8
NB = B // NCORES          # batches per core = 2
TT = 125                  # tokens per tile (4000 = 32*125)
ST = 2000                 # tokens per x-DMA supertile (16 tiles)
NTILES = T // TT          # 32
NSUP = T // ST            # 2
TPS = ST // TT            # tiles per supertile = 16
NSEG = 16                 # segments per 1024 codes
SEGW = K // NSEG          # 64 candidate codes per segment (stride-16)
XR = D + 2                # x rows: 64 features + two ones rows = 66

_compiled = None


def _build_program():
    import concourse.bass as bass
    import concourse.tile as tile
    from concourse import bacc, mybir

    nc = bacc.Bacc(
        "TRN2",
        target_bir_lowering=False,
        debug=False,
        enable_asserts=False,
        num_devices=NCORES,
    )
    f32 = mybir.dt.float32
    f16 = mybir.dt.float16
    xa = nc.dram_tensor("xa", [NB, G, XR, T], f16, kind="ExternalInput").ap()
    et = nc.dram_tensor("et", [G, XR, K], f16, kind="ExternalInput").ap()
    om = nc.dram_tensor(
        "om", [G * NB, TT, NTILES * NSEG], f16, kind="ExternalOutput"
    ).ap()

    def fold(out_flat, in_flat, t, w):
        """Elementwise fp16 max of the two halves of each w-wide block:
        in [p, t*w] -> out [p, t*(w/2)]; pairs element j with j+w/2."""
        ri = in_flat.rearrange("p (t k) -> p t k", t=t, k=w)
        ro = out_flat.rearrange("p (t k) -> p t k", t=t, k=w // 2)
        nc.vector.tensor_max(ro, ri[:, :, :w // 2], ri[:, :, w // 2:])

    with tile.TileContext(nc) as tc, ExitStack() as ctx:
        epool = ctx.enter_context(tc.tile_pool(name="e", bufs=1))
        xpool = ctx.enter_context(tc.tile_pool(name="x", bufs=3))
        ppool = ctx.enter_context(
            tc.tile_pool(name="ps", bufs=2, space=bass.MemorySpace.PSUM)
        )
        spool = ctx.enter_context(tc.tile_pool(name="scp", bufs=3))
        tpool = ctx.enter_context(tc.tile_pool(name="tree", bufs=2))
        mpool = ctx.enter_context(tc.tile_pool(name="mseg", bufs=2))

        etiles = []
        for g in range(G):
            e_t = epool.tile([XR, K], f16, tag=f"e{g}")
            nc.sync.dma_start(e_t[:], et[g])
            etiles.append(e_t)

        for g in range(G):
            for b in range(NB):
                gb = g * NB + b
                # 1 of 8 quads runs DVE-direct from PSUM to offload Act
                direct = {0}
                m_sb = mpool.tile([TT, NTILES * NSEG], f16)
                for s in range(NSUP):
                    xt = xpool.tile([XR, ST], f16, tag="x")
                    nc.sync.dma_start(xt[:], xa[b, g, :, s * ST:(s + 1) * ST])
                    for quad in range(TPS // 4):
                        qg = s * (TPS // 4) + quad
                        tbase = s * TPS + quad * 4
                        if qg not in direct:
                            # Route A: Act stages 4 tiles of PSUM scores to
                            # SBUF fp16, DVE folds a t=4-batched tree.
                            scp = spool.tile([TT, 4 * K], f16)
                            for pr in range(2):
                                ps = ppool.tile([TT, 2 * K], f32)
                                for h in range(2):
                                    k = quad * 4 + pr * 2 + h
                                    xsl = xt[:, k * TT:(k + 1) * TT]
                                    nc.tensor.matmul(
                                        ps[:, h * K:h * K + K // 2], xsl,
                                        etiles[g][:, :K // 2],
                                        start=True, stop=True)
                                    nc.tensor.matmul(
                                        ps[:, h * K + K // 2:(h + 1) * K],
                                        xsl, etiles[g][:, K // 2:],
                                        start=True, stop=True)
                                nc.scalar.activation(
                                    scp[:, pr * 2 * K:(pr + 1) * 2 * K],
                                    ps[:], mybir.ActivationFunctionType.Copy)
                            l1 = tpool.tile([TT, 2048], f16, tag="l1")
                            fold(l1[:], scp[:], 4, 1024)
                            l2 = tpool.tile([TT, 1024], f16, tag="l2")
                            fold(l2[:], l1[:], 4, 512)
                            l3 = tpool.tile([TT, 512], f16, tag="l3")
                            fold(l3[:], l2[:], 4, 256)
                            l4 = tpool.tile([TT, 256], f16, tag="l4")
                            fold(l4[:], l3[:], 4, 128)
                            l5 = tpool.tile([TT, 128], f16, tag="l5")
                            fold(l5[:], l4[:], 4, 64)
                            fold(m_sb[:, tbase * NSEG:(tbase + 4) * NSEG],
                                 l5[:], 4, 32)
                        else:
                            # Route B: DVE folds straight from PSUM (f32 in,
                            # f16 out) per 2-tile pair; no Act involvement.
                            for pr in range(2):
                                ps = ppool.tile([TT, 2 * K], f32)
                                for h in range(2):
                                    k = quad * 4 + pr * 2 + h
                                    xsl = xt[:, k * TT:(k + 1) * TT]
                                    nc.tensor.matmul(
                                        ps[:, h * K:h * K + K // 2], xsl,
                                        etiles[g][:, :K // 2],
                                        start=True, stop=True)
                                    nc.tensor.matmul(
                                        ps[:, h * K + K // 2:(h + 1) * K],
                                        xsl, etiles[g][:, K // 2:],
                                        start=True, stop=True)
                                # DVE TensorTensor may read only one PSUM
                                # input: stage upper halves to SBUF first.
                                ri = ps[:].rearrange("p (t k) -> p t k",
                                                     t=2, k=K)
                                s16u = tpool.tile([TT, 1024], f16, tag="su")
                                rs = s16u[:].rearrange("p (t k) -> p t k",
                                                       t=2, k=K // 2)
                                nc.vector.tensor_copy(rs, ri[:, :, K // 2:])
                                b1 = tpool.tile([TT, 1024], f16, tag="b1")
                                nc.vector.tensor_max(
                                    b1[:].rearrange("p (t k) -> p t k",
                                                    t=2, k=K // 2),
                                    ri[:, :, :K // 2], rs)
                                b2 = tpool.tile([TT, 512], f16, tag="b2")
                                fold(b2[:], b1[:], 2, 512)
                                b3 = tpool.tile([TT, 256], f16, tag="b3")
                                fold(b3[:], b2[:], 2, 256)
                                b4 = tpool.tile([TT, 128], f16, tag="b4")
                                fold(b4[:], b3[:], 2, 128)
                                b5 = tpool.tile([TT, 64], f16, tag="b5")
                                fold(b5[:], b4[:], 2, 64)
                                t0 = tbase + pr * 2
                                fold(m_sb[:, t0 * NSEG:(t0 + 2) * NSEG],
                                     b5[:], 2, 32)
                nc.sync.dma_start(om[gb], m_sb[:])

    nc.compile()
    return nc


def _get_compiled():
    global _compiled
    if _compiled is None:
        _compiled = _build_program()
    return _compiled


def _prep_inputs(x, codebooks):
    # xa: [B, G, 66, T] fp16 — rows 0..63 = fp16(x), rows 64,65 = 1.0
    xg = x.reshape(B, G, D, T)
    xa = np.empty((B, G, XR, T), dtype=np.float16)
    xa[:, :, :D] = xg
    xa[:, :, D:] = 1.0
    # et: [G, 66, K] fp16 — rows 0..63 = 2*E^T; rows 64,65 = -|e|^2 hi/lo
    e2 = (codebooks.astype(np.float32) ** 2).sum(-1)          # [G, K]
    eh = (-e2).astype(np.float16)
    el = (-e2 - eh.astype(np.float32)).astype(np.float16)
    et = np.empty((G, XR, K), dtype=np.float16)
    et[:, :D] = 2.0 * np.transpose(codebooks, (0, 2, 1))
    et[:, D] = eh
    et[:, D + 1] = el
    return xa, et


def run_device(x, codebooks, trace=False):
    from concourse.bass_utils import run_bass_kernel_spmd

    nc = _get_compiled()
    xa, et = _prep_inputs(np.asarray(x, np.float32),
                          np.asarray(codebooks, np.float32))
    in_maps = []
    for core in range(NCORES):
        sl = slice(core * NB, (core + 1) * NB)
        in_maps.append({"xa": np.ascontiguousarray(xa[sl]), "et": et})
    res = run_bass_kernel_spmd(nc, in_maps, list(range(NCORES)), trace=trace)
    return res


# candidate code indices per segment: stride-16 residue classes [NSEG, SEGW]
_CAND = np.arange(NSEG)[:, None] + NSEG * np.arange(SEGW)[None, :]


def _host_finish(x, codebooks, m16):
    """m16: [G, B, T, NSEG] fp32 device segment maxima.
    Rescore the top-2 segments' 2*64 candidates exactly in fp32."""
    xg = x.reshape(B, G, D, T)
    # tokens as [G, B*T, D]
    tok = np.ascontiguousarray(
        np.transpose(xg, (1, 0, 3, 2)).reshape(G, B * T, D))
    m2 = m16.reshape(G, B * T, NSEG)
    # top-2 segments per token
    s1 = np.argmax(m2, axis=-1)                               # [G, N]
    m2m = np.copy(m2)
    np.put_along_axis(m2m, s1[..., None], -np.inf, axis=-1)
    s2 = np.argmax(m2m, axis=-1)                              # [G, N]
    out = np.empty((B, G, D, T), dtype=np.float32)
    n = B * T
    for g in range(G):
        cb = codebooks[g].astype(np.float32)                  # [K, D]
        e2 = (cb * cb).sum(-1)                                # [K]
        w = 2.0 * cb.T                                        # [D, K]
        best_val = np.full(n, -np.inf, dtype=np.float32)
        best_idx = np.zeros(n, dtype=np.int64)
        for seg in range(NSEG):
            cand = _CAND[seg]                                 # [64]
            mask = (s1[g] == seg) | (s2[g] == seg)
            rows = np.nonzero(mask)[0]
            if rows.size == 0:
                continue
            a = tok[g][rows]                                  # [N_s, D]
            sc = a @ w[:, cand] - e2[cand]                    # [N_s, 64]
            loc = np.argmax(sc, axis=1)
            val = sc[np.arange(rows.size), loc]
            idx = cand[loc]
            upd = (val > best_val[rows]) | (
                (val == best_val[rows]) & (idx < best_idx[rows]))
            r_upd = rows[upd]
            best_val[r_upd] = val[upd]
            best_idx[r_upd] = idx[upd]
        q = cb[best_idx]                                      # [N, D]
        out[:, g] = q.reshape(B, T, D).transpose(0, 2, 1)
    return out.reshape(B, C, F, T)


def kernel(x, codebooks):
    x = np.asarray(x, dtype=np.float32)
    codebooks = np.asarray(codebooks, dtype=np.float32)
    res = run_device(x, codebooks)
    # om [G*NB, TT, NTILES*NSEG] fp16; token t = tloc*TT + p
    m16 = np.empty((G, B, T, NSEG), dtype=np.float32)
    for core in range(NCORES):
        o = res.results[core]["om"].astype(np.float32).reshape(
            G, NB, TT, NTILES, NSEG)
        m16[:, core * NB:(core + 1) * NB] = o.transpose(0, 1, 3, 2, 4).reshape(
            G, NB, T, NSEG
        )
    q = _host_finish(x, codebooks, m16)
    x_q = x + (q - x)
    return x_q, q
